# revision 23
# baseline (speedup 1.0000x reference)
"""LoRA linear layer on 8 Trainium2 NeuronCores.

Computes out = x @ (lora_B @ lora_A * 2).T + bias for
x [4, 2048, 4096], lora_A [16, 4096], lora_B [4096, 16], bias [4096].

Strategy: pure data parallel — shard x over batch*seq (8192 rows -> 1024
rows/core), replicate the tiny LoRA weights. Per core, exploit the rank-16
structure: y = x @ A^T (contract 4096), z = y @ B^T * 2 + bias (contract 16).

Memory-regime tuning (final):
  * All device-side matmul traffic is bf16 (1 PE cycle/row vs 4 for fp32;
    rel-err budget is 2e-2, bf16 lands ~5e-3). Output stored bf16 and
    upcast on the host. Per-core HBM traffic: 8 MiB in + 8 MiB out.
  * Host pre-transposes each x shard to x^T and pre-tiles it so each
    512 KiB load piece is a fully contiguous [128, 4x512] DRAM slab
    (4 KiB per partition line -> line-rate descriptors). No PE
    transposes, no transpose PSUM round-trip.
  * SDMA engines round-robin between all in-flight transfers at packet
    granularity, so a deep prefetch queue delays the FIRST transfer's
    completion. Loads are issued on the SP HWDGE ring in exact consume
    order with a 6-piece pool bound (~3 MiB in flight, arrivals every
    ~1.5 us keep PE stalls under the HAM re-throttle window); stores +
    const loads go on the separate ACT ring so they never
    head-of-line-block a load issue.
  * 512-row blocks -> 64 GEMM1 + 64 GEMM2 matmuls, all N=512 (per-MM
    issue/LDWEIGHTS overhead amortized).
  * The PE HAM clock-gate only holds 2.4 GHz under high array activity;
    K=17 GEMM2 matmuls streamed at 1.2 GHz. BB and yt are therefore
    zero/one-padded to the full 128 contraction rows (zero rows kill
    the garbage), so every matmul streams the full array; warm-up
    matmuls cover the initial load latency.
  * Block 0's GEMM2 tiles interleave with block 1's loads/GEMM1 so y(b1)
    completes right after its last piece lands; PSUM -> SBUF drain of z
    alternates Vector/Scalar (4.5/3.5 average split; ScalarE also
    issues the stores) over a 7-bank rotation, and both y accumulators
    share one PSUM bank.
"""

import sys

import numpy as np

if "/opt/trn_rl_repo" not in sys.path:
    sys.path.insert(0, "/opt/trn_rl_repo")

import ml_dtypes

import concourse.bass as bass
import concourse.mybir as mybir
from concourse import bacc
from concourse.bass_utils import run_bass_kernel_spmd
from concourse.tile import TileContext

N_CORES = 8
B, S, IN_F, OUT_F, R = 4, 2048, 4096, 4096, 16
ROWS = B * S // N_CORES  # 1024 rows per core
SCALING = 2.0  # alpha / r = 32 / 16
FP32 = mybir.dt.float32
BF16 = mybir.dt.bfloat16
BF = ml_dtypes.bfloat16
P = 128
NK = IN_F // P  # 32 contraction chunks for GEMM1
RB = 512  # rows per block (one PSUM bank of fp32 y^T)
NB = ROWS // RB  # 2 blocks per core
KQ = 4  # k-chunks per load piece
NQ = NK // KQ  # 8 load pieces per block
SCOLS = KQ * RB  # 2048 piece columns
HT = RB // P  # 4 row-tiles per block
ZC = 512  # GEMM2 moving chunk (one PSUM bank of fp32)
NJ = OUT_F // ZC  # 8 output chunks per row tile
NWARM = 10  # HAM warm-up matmuls (~4 us cold — one SHORT window)

_nc_cache = None


def build_nc() -> bass.Bass:
    nc = bacc.Bacc()
    # x^T pieces: piece (b, q) at rows [(b*NQ+q)*P, +P), fully contiguous.
    xt_d = nc.declare_dram_parameter(
        "xt", [NB * NQ * P, SCOLS], BF16, isOutput=False
    )
    # (2A)^T, partition-major chunk tiling: [128, 32*16]
    at_d = nc.declare_dram_parameter("at", [P, NK * R], BF16, isOutput=False)
    bb_d = nc.declare_dram_parameter("bb", [R + 1, OUT_F], BF16, isOutput=False)
    out_d = nc.declare_dram_parameter("out", [ROWS, OUT_F], BF16, isOutput=True)

    with TileContext(nc) as tc:
        with (
            tc.tile_pool(name="const", bufs=1) as const,
            tc.tile_pool(name="xin", bufs=6) as xin,
            tc.tile_pool(name="ytp", bufs=2) as ytp,
            tc.tile_pool(name="zrp", bufs=4) as zrp,
            tc.tile_pool(name="ypsum", bufs=1, space="PSUM") as ypsum,
            tc.tile_pool(name="zpsum", bufs=7, space="PSUM") as zpsum,
        ):
            # HAM warm-up: keep the PE busy through one full activity window
            # while the first x piece loads, so real matmuls run at full
            # clock. Reuses the z PSUM rotation — no extra bank.
            wsrc = const.tile([P, ZC], BF16)
            nc.vector.memset(wsrc[:, :], 0.0)
            w_ps = zpsum.tile([P, ZC], FP32, tag="zz")
            for _ in range(NWARM):
                nc.tensor.matmul(
                    w_ps, lhsT=wsrc[:, :P], rhs=wsrc[:, :], start=True, stop=True
                )

            # Const loads on the ACT ring — keep the SP ring free for x.
            at_sb = const.tile([P, NK * R], BF16)
            nc.scalar.dma_start(out=at_sb[:, :], in_=at_d[:, :])
            # BB zero-padded to 128 contraction rows: full-array GEMM2
            # matmuls keep the HAM activity monitor at the 2.4 GHz clock.
            bb = const.tile([P, OUT_F], BF16)
            nc.vector.memset(bb[:, :], 0.0)
            nc.scalar.dma_start(out=bb[0 : R + 1, :], in_=bb_d[:, :])

            piece_idx = [0]

            def load_piece():
                i = piece_idx[0]
                piece_idx[0] += 1
                xt_p = xin.tile([P, SCOLS], BF16, tag="x")
                nc.sync.dma_start(
                    out=xt_p[:, :], in_=xt_d[i * P : (i + 1) * P, :]
                )
                return xt_p

            def g1_group(y_ps, piece, q):
                for kk in range(KQ):
                    k = q * KQ + kk
                    nc.tensor.matmul(
                        y_ps,
                        lhsT=at_sb[:, k * R : (k + 1) * R],
                        rhs=piece[:, kk * RB : (kk + 1) * RB],
                        start=(k == 0),
                        stop=(k == NK - 1),
                    )

            def make_yt(y_ps):
                # Ones-fill the whole 128-row tile (row 16 keeps the 1.0 for
                # the bias trick; rows 17+ meet zero BB rows), then overwrite
                # rows 0:16 with y.
                yt_sb = ytp.tile([P, RB], BF16, tag="yt")
                nc.vector.memset(yt_sb[:, :], 1.0)
                nc.scalar.copy(out=yt_sb[0:R, :], in_=y_ps)
                return yt_sb

            def g2_tile(yt_sb, rt, h):
                zrow = zrp.tile([P, OUT_F], BF16, tag="z")
                for j in range(NJ):
                    z_ps = zpsum.tile([P, ZC], FP32, tag="zz")
                    nc.tensor.matmul(
                        z_ps,
                        lhsT=yt_sb[:, h * P : (h + 1) * P],
                        rhs=bb[:, j * ZC : (j + 1) * ZC],
                        start=True,
                        stop=True,
                    )
                    dst = zrow[:, j * ZC : (j + 1) * ZC]
                    if j % 2 == 0 or (j == 7 and rt % 2 == 0):
                        nc.vector.tensor_copy(out=dst, in_=z_ps)
                    else:
                        nc.scalar.copy(out=dst, in_=z_ps)
                nc.scalar.dma_start(
                    out=out_d[rt * P : (rt + 1) * P, :], in_=zrow[:, :]
                )

            # Block 0: load + GEMM1.
            p0 = [load_piece() for _ in range(NQ)]
            y0 = ypsum.tile([R, RB], FP32, tag="y")
            for q in range(NQ):
                g1_group(y0, p0[q], q)
            yt0 = make_yt(y0)

            # Interleave block 0's GEMM2 tiles with block 1's loads/GEMM1 so
            # y(b1) completes right after its last piece lands and the final
            # stores start as early as possible.
            y1 = ypsum.tile([R, RB], FP32, tag="y")
            for q in range(NQ):
                p1q = load_piece()
                if q % 2 == 0:
                    g2_tile(yt0, q // 2, q // 2)
                g1_group(y1, p1q, q)
            yt1 = make_yt(y1)
            for h in range(HT):
                g2_tile(yt1, HT + h, h)

    nc.finalize()  # Bacc.finalize runs compile(): wait legalization + reg alloc
    return nc


def make_in_maps(x, lora_A, lora_B, bias):
    x2 = np.asarray(x, dtype=np.float32).reshape(B * S, IN_F)
    # (2A)^T [4096, 16] -> partition-major chunk tiling [128, 32*16]
    at = (np.asarray(lora_A, dtype=np.float32).T * SCALING).astype(BF)
    at = np.ascontiguousarray(
        at.reshape(NK, P, R).transpose(1, 0, 2).reshape(P, NK * R)
    )
    bbh = np.ascontiguousarray(
        np.concatenate(
            [
                np.asarray(lora_B, dtype=np.float32).T,
                np.asarray(bias, dtype=np.float32)[None, :],
            ],
            axis=0,
        ).astype(BF)
    )
    xb = x2.astype(BF)
    maps = []
    for s in np.split(xb, N_CORES, axis=0):
        # s [1024 rows, 4096] -> x^T [4096 = (q kk p), 1024 = (b r)]
        # -> piece-major [(b q p), (kk r)]
        xt = np.ascontiguousarray(
            s.T.reshape(NQ, KQ, P, NB, RB)
            .transpose(3, 0, 2, 1, 4)
            .reshape(NB * NQ * P, SCOLS)
        )
        maps.append({"xt": xt, "at": at, "bb": bbh})
    return maps


def run(inputs: dict, trace: bool = False, **kw):
    global _nc_cache
    if _nc_cache is None:
        _nc_cache = build_nc()
    in_maps = make_in_maps(**inputs)
    res = run_bass_kernel_spmd(
        _nc_cache, in_maps, list(range(N_CORES)), trace=trace, **kw
    )
    out = (
        np.concatenate([res.results[i]["out"] for i in range(N_CORES)], axis=0)
        .astype(np.float32)
        .reshape(B, S, OUT_F)
    )
    return out, res


def kernel(**inputs) -> np.ndarray:
    out, _ = run(inputs)
    return out


# revision 24
# speedup vs baseline: 1.0335x; 1.0335x over previous
"""LoRA linear layer on 8 Trainium2 NeuronCores.

Computes out = x @ (lora_B @ lora_A * 2).T + bias for
x [4, 2048, 4096], lora_A [16, 4096], lora_B [4096, 16], bias [4096].

Strategy: pure data parallel — shard x over batch*seq (8192 rows -> 1024
rows/core), replicate the tiny LoRA weights. Per core, exploit the rank-16
structure: y = x @ A^T (contract 4096), z = y @ B^T * 2 + bias (contract 16).

Memory-regime tuning (final):
  * All device-side matmul traffic is bf16 (1 PE cycle/row vs 4 for fp32;
    rel-err budget is 2e-2, bf16 lands ~5e-3). Output stored bf16 and
    upcast on the host. Per-core HBM traffic: 8 MiB in + 8 MiB out.
  * Host pre-transposes each x shard to x^T and pre-tiles it so each
    512 KiB load piece is a fully contiguous [128, 4x512] DRAM slab
    (4 KiB per partition line -> line-rate descriptors). No PE
    transposes, no transpose PSUM round-trip.
  * SDMA engines round-robin between all in-flight transfers at packet
    granularity, so a deep prefetch queue delays the FIRST transfer's
    completion. Loads are issued on the SP HWDGE ring in exact consume
    order with a 6-piece pool bound (~3 MiB in flight, arrivals every
    ~1.5 us keep PE stalls under the HAM re-throttle window); stores +
    const loads go on the separate ACT ring so they never
    head-of-line-block a load issue.
  * 512-row blocks -> 64 GEMM1 + 64 GEMM2 matmuls, all N=512 (per-MM
    issue/LDWEIGHTS overhead amortized).
  * The PE HAM clock-gate only holds 2.4 GHz under high array activity;
    K=17 GEMM2 matmuls streamed at 1.2 GHz. BB and yt are therefore
    zero/one-padded to the full 128 contraction rows (zero rows kill
    the garbage), so every matmul streams the full array; warm-up
    matmuls cover the initial load latency.
  * Block 0's GEMM2 tiles interleave with block 1's loads/GEMM1 so y(b1)
    completes right after its last piece lands; PSUM -> SBUF drain of z
    alternates Vector/Scalar (4.5/3.5 average split; ScalarE also
    issues the stores) over a 7-bank rotation, and both y accumulators
    share one PSUM bank.
"""

import sys

import numpy as np

if "/opt/trn_rl_repo" not in sys.path:
    sys.path.insert(0, "/opt/trn_rl_repo")

import ml_dtypes

import concourse.bass as bass
import concourse.mybir as mybir
from concourse import bacc
from concourse.bass_utils import run_bass_kernel_spmd
from concourse.tile import TileContext

N_CORES = 8
B, S, IN_F, OUT_F, R = 4, 2048, 4096, 4096, 16
ROWS = B * S // N_CORES  # 1024 rows per core
SCALING = 2.0  # alpha / r = 32 / 16
FP32 = mybir.dt.float32
BF16 = mybir.dt.bfloat16
BF = ml_dtypes.bfloat16
P = 128
NK = IN_F // P  # 32 contraction chunks for GEMM1
RB = 512  # rows per block (one PSUM bank of fp32 y^T)
NB = ROWS // RB  # 2 blocks per core
KQ = 4  # k-chunks per load piece
NQ = NK // KQ  # 8 load pieces per block
SCOLS = KQ * RB  # 2048 piece columns
HT = RB // P  # 4 row-tiles per block
ZC = 512  # GEMM2 moving chunk (one PSUM bank of fp32)
NJ = OUT_F // ZC  # 8 output chunks per row tile
NWARM = 10  # HAM warm-up matmuls (~4 us cold — one SHORT window)

_nc_cache = None


def build_nc() -> bass.Bass:
    nc = bacc.Bacc()
    # x^T pieces: piece (b, q) at rows [(b*NQ+q)*P, +P), fully contiguous.
    xt_d = nc.declare_dram_parameter(
        "xt", [NB * NQ * P, SCOLS], BF16, isOutput=False
    )
    # (2A)^T, partition-major chunk tiling: [128, 32*16]
    at_d = nc.declare_dram_parameter("at", [P, NK * R], BF16, isOutput=False)
    bb_d = nc.declare_dram_parameter("bb", [R + 1, OUT_F], BF16, isOutput=False)
    out_d = nc.declare_dram_parameter("out", [ROWS, OUT_F], BF16, isOutput=True)

    with TileContext(nc) as tc:
        with (
            tc.tile_pool(name="const", bufs=1) as const,
            tc.tile_pool(name="xin", bufs=6) as xin,
            tc.tile_pool(name="ytp", bufs=2) as ytp,
            tc.tile_pool(name="zrp", bufs=4) as zrp,
            tc.tile_pool(name="ypsum", bufs=1, space="PSUM") as ypsum,
            tc.tile_pool(name="zpsum", bufs=7, space="PSUM") as zpsum,
        ):
            # HAM warm-up: keep the PE busy through one full activity window
            # while the first x piece loads, so real matmuls run at full
            # clock. Reuses the z PSUM rotation — no extra bank.
            wsrc = const.tile([P, ZC], BF16)
            nc.vector.memset(wsrc[:, :], 0.0)
            w_ps = zpsum.tile([P, ZC], FP32, tag="zz")
            for _ in range(NWARM):
                nc.tensor.matmul(
                    w_ps, lhsT=wsrc[:, :P], rhs=wsrc[:, :], start=True, stop=True
                )

            # Const loads on the ACT ring — keep the SP ring free for x.
            at_sb = const.tile([P, NK * R], BF16)
            nc.scalar.dma_start(out=at_sb[:, :], in_=at_d[:, :])
            # BB zero-padded to 128 contraction rows: full-array GEMM2
            # matmuls keep the HAM activity monitor at the 2.4 GHz clock.
            bb = const.tile([P, OUT_F], BF16)
            nc.vector.memset(bb[:, :], 0.0)
            nc.scalar.dma_start(out=bb[0 : R + 1, :], in_=bb_d[:, :])

            piece_idx = [0]

            def load_piece():
                i = piece_idx[0]
                piece_idx[0] += 1
                xt_p = xin.tile([P, SCOLS], BF16, tag="x")
                nc.sync.dma_start(
                    out=xt_p[:, :], in_=xt_d[i * P : (i + 1) * P, :]
                )
                return xt_p

            def g1_group(y_ps, piece, q):
                for kk in range(KQ):
                    k = q * KQ + kk
                    nc.tensor.matmul(
                        y_ps,
                        lhsT=at_sb[:, k * R : (k + 1) * R],
                        rhs=piece[:, kk * RB : (kk + 1) * RB],
                        start=(k == 0),
                        stop=(k == NK - 1),
                    )

            def make_yt(y_ps):
                # Ones-fill the whole 128-row tile (row 16 keeps the 1.0 for
                # the bias trick; rows 17+ meet zero BB rows), then overwrite
                # rows 0:16 with y.
                yt_sb = ytp.tile([P, RB], BF16, tag="yt")
                nc.vector.memset(yt_sb[:, :], 1.0)
                nc.scalar.copy(out=yt_sb[0:R, :], in_=y_ps)
                return yt_sb

            def g2_tile(yt_sb, rt, h):
                zrow = zrp.tile([P, OUT_F], BF16, tag="z")
                for j in range(NJ):
                    z_ps = zpsum.tile([P, ZC], FP32, tag="zz")
                    nc.tensor.matmul(
                        z_ps,
                        lhsT=yt_sb[:, h * P : (h + 1) * P],
                        rhs=bb[:, j * ZC : (j + 1) * ZC],
                        start=True,
                        stop=True,
                    )
                    dst = zrow[:, j * ZC : (j + 1) * ZC]
                    if j % 2 == 0 or (j == 7 and rt % 2 == 0):
                        nc.vector.tensor_copy(out=dst, in_=z_ps)
                    else:
                        nc.scalar.copy(out=dst, in_=z_ps)
                nc.scalar.dma_start(
                    out=out_d[rt * P : (rt + 1) * P, :], in_=zrow[:, :]
                )

            # Block 0: load + GEMM1. Piece arrivals pace the later GEMM1
            # groups; full-array filler matmuls keep the HAM activity
            # monitor warm through those waits so GEMM2 starts at 2.4 GHz.
            p0 = [load_piece() for _ in range(NQ)]
            y0 = ypsum.tile([R, RB], FP32, tag="y")
            for q in range(NQ):
                g1_group(y0, p0[q], q)
                if 3 <= q < NQ - 1:
                    for _ in range(4):
                        nc.tensor.matmul(
                            w_ps,
                            lhsT=wsrc[:, :P],
                            rhs=wsrc[:, :],
                            start=True,
                            stop=True,
                        )
            yt0 = make_yt(y0)

            # Interleave block 0's GEMM2 tiles with block 1's loads/GEMM1 so
            # y(b1) completes right after its last piece lands and the final
            # stores start as early as possible.
            y1 = ypsum.tile([R, RB], FP32, tag="y")
            for q in range(NQ):
                p1q = load_piece()
                if q % 2 == 0:
                    g2_tile(yt0, q // 2, q // 2)
                g1_group(y1, p1q, q)
            yt1 = make_yt(y1)
            for h in range(HT):
                g2_tile(yt1, HT + h, h)

    nc.finalize()  # Bacc.finalize runs compile(): wait legalization + reg alloc
    return nc


def make_in_maps(x, lora_A, lora_B, bias):
    x2 = np.asarray(x, dtype=np.float32).reshape(B * S, IN_F)
    # (2A)^T [4096, 16] -> partition-major chunk tiling [128, 32*16]
    at = (np.asarray(lora_A, dtype=np.float32).T * SCALING).astype(BF)
    at = np.ascontiguousarray(
        at.reshape(NK, P, R).transpose(1, 0, 2).reshape(P, NK * R)
    )
    bbh = np.ascontiguousarray(
        np.concatenate(
            [
                np.asarray(lora_B, dtype=np.float32).T,
                np.asarray(bias, dtype=np.float32)[None, :],
            ],
            axis=0,
        ).astype(BF)
    )
    xb = x2.astype(BF)
    maps = []
    for s in np.split(xb, N_CORES, axis=0):
        # s [1024 rows, 4096] -> x^T [4096 = (q kk p), 1024 = (b r)]
        # -> piece-major [(b q p), (kk r)]
        xt = np.ascontiguousarray(
            s.T.reshape(NQ, KQ, P, NB, RB)
            .transpose(3, 0, 2, 1, 4)
            .reshape(NB * NQ * P, SCOLS)
        )
        maps.append({"xt": xt, "at": at, "bb": bbh})
    return maps


def run(inputs: dict, trace: bool = False, **kw):
    global _nc_cache
    if _nc_cache is None:
        _nc_cache = build_nc()
    in_maps = make_in_maps(**inputs)
    res = run_bass_kernel_spmd(
        _nc_cache, in_maps, list(range(N_CORES)), trace=trace, **kw
    )
    out = (
        np.concatenate([res.results[i]["out"] for i in range(N_CORES)], axis=0)
        .astype(np.float32)
        .reshape(B, S, OUT_F)
    )
    return out, res


def kernel(**inputs) -> np.ndarray:
    out, _ = run(inputs)
    return out
